# revision 1
# baseline (speedup 1.0000x reference)
"""Trainium2 Bass kernel for the DJconv hypergraph message-passing layer.

Reference computation (per full input):
    gram = H.T @ H                              [E, E]
    Hu   = concat([H, H @ gram], 1) >= 0.5      [N, 2E] binary
    dv   = Hu.sum(1);  inv = rsqrt(dv) (0 where dv==0)
    out  = ((1 + inv)[:, None] * U) @ weight + bias

Sharding: rows (nodes) split across 8 NeuronCores; the [E, E] gram is
all-reduced on device; weight/bias replicated.
"""

import numpy as np
import ml_dtypes

import concourse.bass as bass
import concourse.tile as tile
from concourse import bacc, mybir
from concourse.bass_utils import run_bass_kernel_spmd

F32 = mybir.dt.float32
F32R = mybir.dt.float32r
BF16 = mybir.dt.bfloat16
FP8 = mybir.dt.float8e4

N_FULL, E, IN_C, OUT_C = 131072, 256, 128, 256
NCORES = 8
ROWS = N_FULL // NCORES          # 16384 rows per core
P = 128


def build_program(rows=ROWS, ncores=NCORES):
    """Build + compile the SPMD single-core program (same NEFF on all cores)."""
    assert rows % 512 == 0
    nt = rows // P          # node tiles per core
    ns = nt // 4            # super tiles (4 node tiles each)

    nc = bacc.Bacc("TRN2", target_bir_lowering=False, debug=False,
                   num_devices=ncores)

    H = nc.dram_tensor("H", [rows, E], F32, kind="ExternalInput").ap()
    U = nc.dram_tensor("U", [rows, IN_C], F32, kind="ExternalInput").ap()
    W = nc.dram_tensor("W", [IN_C, OUT_C], F32, kind="ExternalInput").ap()
    BIASB = nc.dram_tensor("BIASB", [P, OUT_C], F32, kind="ExternalInput").ap()
    ID16 = nc.dram_tensor("ID16", [P, P], BF16, kind="ExternalInput").ap()
    ID32 = nc.dram_tensor("ID32", [P, P], F32, kind="ExternalInput").ap()
    OUT = nc.dram_tensor("OUT", [rows, OUT_C], F32, kind="ExternalOutput").ap()

    # super-tile views: node (s*512 + p*4 + j) — consecutive nodes stay on one
    # partition so each DMA descriptor covers 4 rows (4KB for H) contiguously.
    # The permutation is applied identically to H, U and OUT, so the kernel is
    # self-consistent (gram sums over all nodes; everything else is per-node).
    H_r = H.rearrange("(s p j) e -> s p j e", j=4, p=P)
    U_r = U.rearrange("(s p j) c -> s p j c", j=4, p=P)
    OUT_r = OUT.rearrange("(s p j) o -> s p j o", j=4, p=P)

    with tile.TileContext(nc) as tc:
        _body(tc, nt, ns, H_r, U_r, OUT_r, W, BIASB, ID16, ID32)

    nc.compile()
    return nc


def _body(tc, nt, ns, H_r, U_r, OUT_r, W, BIASB, ID16, ID32):
    nc = tc.nc
    Add = mybir.AluOpType.add
    Mult = mybir.AluOpType.mult
    IsGe = mybir.AluOpType.is_ge
    AF = mybir.ActivationFunctionType

    import contextlib
    ctx = contextlib.ExitStack()
    with ctx:
        const = ctx.enter_context(tc.tile_pool(name="const", bufs=1))
        htst = ctx.enter_context(tc.tile_pool(name="htstore", bufs=1))
        work = ctx.enter_context(tc.tile_pool(name="work", bufs=1))
        upool = ctx.enter_context(tc.tile_pool(name="uload", bufs=4))
        opool = ctx.enter_context(tc.tile_pool(name="ost", bufs=4))
        scr = ctx.enter_context(tc.tile_pool(name="scratch", bufs=3))
        dram = ctx.enter_context(tc.tile_pool(name="dram", bufs=1, space="DRAM"))

        # ---- constants ----
        id16 = const.tile([P, P], BF16)
        nc.sync.dma_start(id16[:], ID16[:])
        id32 = const.tile([P, P], F32)
        nc.sync.dma_start(id32[:], ID32[:])
        w_sb = const.tile([IN_C, OUT_C], F32)
        nc.sync.dma_start(w_sb[:], W[:])
        bias_b = const.tile([P, OUT_C], F32)
        nc.sync.dma_start(bias_b[:], BIASB[:])
        neghalf = const.tile([P, 1], F32)
        nc.vector.memset(neghalf[:], -0.5 / 64)

        # persistent H^T (feature-major H) in fp8 (H is 0/1: exact), packed as
        # [q, t, n] with edge f = t*128 + q so pass B runs fp8 DoubleRow (K=256
        # in a single matmul). Slot strides are 16-byte aligned as DR requires.
        HTE = htst.tile([P, 2, nt * P], FP8, tag="hte")

        # all of H stays resident (bf16) so transposes can fill the collective
        # window; its pool closes after the transposes so UT can reuse the SBUF
        with tc.tile_pool(name="hallp", bufs=1) as hallp:
            HALL = hallp.tile([P, ns, 4, E], BF16, tag="hall")

            # ---- phase A: stream H, gram triangle; one all-gather (two
            # serialized collectives cost ~2x the ~40us latency floor here).
            # (bf16 payload: gram is only ever thresholded, rounding is safe)
            with tc.tile_pool(name="psA", bufs=1, space="PSUM") as psA:
                gA = psA.tile([P, E], F32, tag="gA")
                gB = psA.tile([P, P], F32, tag="gB")
                for s in range(ns):
                    nc.gpsimd.dma_start(HALL[:, s, :, :], H_r[s])  # f32->bf16
                    for j in range(4):
                        first = (s == 0 and j == 0)
                        last = (s == ns - 1 and j == 3)
                        nc.tensor.matmul(gA[:], HALL[:, s, j, 0:P],
                                         HALL[:, s, j, :],
                                         start=first, stop=last)
                        nc.tensor.matmul(gB[:], HALL[:, s, j, P:E],
                                         HALL[:, s, j, P:E],
                                         start=first, stop=last)
                gcat = work.tile([P, E + P], BF16, tag="gcat")
                nc.vector.tensor_copy(gcat[:, 0:E], gA[:])
                nc.vector.tensor_copy(gcat[:, E:E + P], gB[:])
            cc_in = dram.tile([P, E + P], BF16)
            cc_out = dram.tile([NCORES * P, E + P], BF16)
            nc.sync.dma_start(cc_in[:], gcat[:])
            nc.gpsimd.collective_compute(
                "AllGather", mybir.AluOpType.bypass,
                replica_groups=[list(range(NCORES))],
                ins=[cc_in.opt()],
                outs=[cc_out.opt()],
            )
            gparts = work.tile([P, NCORES, E + P], BF16, tag="gparts")
            nc.sync.dma_start(gparts[:], cc_out[:].rearrange("(r p) f -> p r f", p=P))

            # ---- H^T transposes (overlap the collective) ----
            with tc.tile_pool(name="psT", bufs=3, space="PSUM") as psT:
                for s in range(ns):
                    pt0 = psT.tile([P, 4 * P], BF16, tag="t0")
                    pt1 = psT.tile([P, 4 * P], BF16, tag="t1")
                    for j in range(4):
                        nc.tensor.transpose(pt0[:, j * P:(j + 1) * P],
                                            HALL[:, s, j, 0:P], id16[:])
                        nc.tensor.transpose(pt1[:, j * P:(j + 1) * P],
                                            HALL[:, s, j, P:E], id16[:])
                    sl = slice(s * 4 * P, (s + 1) * 4 * P)
                    nc.vector.tensor_copy(HTE[:, 0, sl], pt0[:])
                    nc.scalar.copy(HTE[:, 1, sl], pt1[:])

        # ---- U^T staging: all tiles transposed up front (raw U; scale follows
        # the matmul) so the final loop is matmul+epilogue only ----
        utp = ctx.enter_context(tc.tile_pool(name="utp", bufs=1))
        UT = utp.tile([P, nt * IN_C], F32, tag="ut")
        with tc.tile_pool(name="psU", bufs=3, space="PSUM") as psU:
            for s in range(ns):
                with tc.tile_wait_until(0.03):
                    ut = upool.tile([P, 4, IN_C], F32, tag="u")
                    nc.sync.dma_start(ut[:], U_r[s])
                pp = psU.tile([P, 4 * IN_C], F32, tag="pp")
                for j in range(4):
                    nc.tensor.transpose(pp[:, j * IN_C:(j + 1) * IN_C],
                                        ut[:, j, :], id32[:])
                if s % 4 != 3:
                    nc.vector.tensor_copy(UT[:, s * 4 * IN_C:(s + 1) * 4 * IN_C], pp[:])
                else:
                    nc.scalar.copy(UT[:, s * 4 * IN_C:(s + 1) * 4 * IN_C], pp[:])

        # tree-sum the 8 gathered gram partials -> [P, F]
        gsum = work.tile([P, E + P], BF16, tag="gsum")
        g4 = work.tile([P, 4, E + P], BF16, tag="g4")
        nc.vector.tensor_tensor(g4[:], gparts[:, 0:4, :], gparts[:, 4:8, :], op=Add)
        g2 = work.tile([P, 2, E + P], BF16, tag="g2")
        nc.vector.tensor_tensor(g2[:], g4[:, 0:2, :], g4[:, 2:4, :], op=Add)
        nc.vector.tensor_tensor(gsum[:], g2[:, 0, :], g2[:, 1, :], op=Add)

        # ---- phase B: HG tiles, threshold counts ----
        dvS = work.tile([P, nt], F32, tag="dvS")   # per-tile accumulated counts
        dvH = work.tile([P, nt], F32, tag="dvH")   # rowsum(H) per tile
        s1p = work.tile([P, nt], F32, tag="s1p")   # 1 + rsqrt(dv)
        with tc.tile_pool(name="psB", bufs=6, space="PSUM") as psB, \
             tc.tile_pool(name="psG", bufs=1, space="PSUM") as psG:
            GW = 272  # 257 padded to a 16B multiple for DoubleRow
            gxp = const.tile([P, 2, GW], FP8, tag="gxp")
            nc.vector.memset(gxp[:], 0.0)
            nc.vector.tensor_scalar(gxp[:, 0, 0:E], gsum[:, 0:E], 1.0 / 64, None,
                                    op0=Mult)
            nc.vector.tensor_scalar(gxp[:, 1, P:E], gsum[:, E:E + P], 1.0 / 64, None,
                                    op0=Mult)
            pgt = psG.tile([P, P], BF16, tag="pgt")
            nc.tensor.transpose(pgt[:], gsum[:, P:E], id16[:])
            nc.vector.tensor_scalar(gxp[:, 1, 0:P], pgt[:], 1.0 / 64, None, op0=Mult)
            nc.vector.memset(gxp[:, 0, E:E + 1], 1.0)
            nc.vector.memset(gxp[:, 1, E:E + 1], 1.0)

            def dv_chunk(c0, c1):
                csl = slice(c0, c1)
                m = c1 - c0
                # counts: even cols hold 2*cnt-256 (Sign), odd cols hold cnt
                nc.vector.tensor_scalar(dvS[:, c0:c1:2], dvS[:, c0:c1:2], 0.5,
                                        float(E) / 2, op0=Mult, op1=Add)
                dv = work.tile([P, nt], F32, tag="dv")
                nc.vector.tensor_tensor(dv[:, csl], dvS[:, csl], dvH[:, csl], op=Add)
                mx = work.tile([P, nt], F32, tag="mx")
                nc.vector.tensor_scalar_max(mx[:, csl], dv[:, csl], 1.0)
                rc = work.tile([P, nt], F32, tag="rc")
                nc.vector.reciprocal(rc[:, csl], mx[:, csl])
                sq = work.tile([P, nt], F32, tag="sq")
                nc.scalar.sqrt(sq[:, csl], dv[:, csl])
                r0 = work.tile([P, nt], F32, tag="r0")
                nc.vector.tensor_tensor(r0[:, csl], sq[:, csl], rc[:, csl], op=Mult)
                q = work.tile([P, nt], F32, tag="q")
                nc.vector.tensor_tensor(q[:, csl], r0[:, csl], r0[:, csl], op=Mult)
                nc.vector.tensor_tensor(q[:, csl], q[:, csl], dv[:, csl], op=Mult)
                nc.vector.tensor_scalar(q[:, csl], q[:, csl], -0.5, 1.5,
                                        op0=Mult, op1=Add)
                nc.vector.tensor_tensor(s1p[:, csl], r0[:, csl], q[:, csl], op=Mult)
                nc.vector.tensor_scalar_add(s1p[:, csl], s1p[:, csl], 1.0)

            CHUNK = min(32, nt)
            for k in range(nt):
                pb = psB.tile([P, GW], F32, tag="pb")
                ksl = slice(k * P, (k + 1) * P)
                nc.tensor.matmul(pb[:], HTE[:, :, ksl], gxp[:],
                                 perf_mode=mybir.MatmulPerfMode.DoubleRow,
                                 start=True, stop=True)
                sg = scr.tile([P, E], BF16, tag="sg")
                if k % 2 == 0:
                    # ACT: sum of sign(HG-0.5) = 2*cnt-256, fixed up below
                    nc.scalar.activation(sg[:], pb[:, 0:E], AF.Sign,
                                         bias=neghalf[:], scale=1.0,
                                         accum_out=dvS[:, k:k + 1])
                else:
                    # DVE: direct count of (HG >= 0.5)
                    nc.vector.tensor_scalar(sg[:], pb[:, 0:E], 0.5 / 64, 0.0,
                                            op0=IsGe, op1=Add,
                                            accum_out=dvS[:, k:k + 1])
                nc.vector.tensor_copy(dvH[:, k:k + 1], pb[:, E:E + 1])
                if (k + 1) % CHUNK == 0:
                    dv_chunk(k + 1 - CHUNK, k + 1)

        # ---- final: out = (1+r) * (U @ W) + bias ----
        with tc.tile_pool(name="psF", bufs=4, space="PSUM") as psF:
            for s in range(ns):
                ob = opool.tile([P, 4, OUT_C], F32, tag="o")
                for j in range(4):
                    k = 4 * s + j
                    po = psF.tile([P, OUT_C], F32, tag="po")
                    nc.tensor.matmul(po[:], UT[:, k * IN_C:(k + 1) * IN_C],
                                     w_sb[:], start=True, stop=True)
                    ys = scr.tile([P, OUT_C], F32, tag="ys")
                    if k % 3 != 2:
                        nc.scalar.mul(ys[:], po[:], s1p[:, k:k + 1])
                    else:
                        nc.vector.tensor_scalar(ys[:], po[:], s1p[:, k:k + 1],
                                                None, op0=Mult)
                    nc.vector.tensor_tensor(ob[:, j, :], ys[:], bias_b[:], op=Add)
                nc.sync.dma_start(OUT_r[s], ob[:])


_CACHE = {}


def _get_program(rows=ROWS):
    if rows not in _CACHE:
        _CACHE[rows] = build_program(rows=rows)
    return _CACHE[rows]


def _make_aux():
    id16 = np.eye(P, dtype=ml_dtypes.bfloat16)
    id32 = np.eye(P, dtype=np.float32)
    return id16, id32


def kernel(H, U, weight, bias, _rows=ROWS, _trace=False):
    H = np.ascontiguousarray(H, dtype=np.float32)
    U = np.ascontiguousarray(U, dtype=np.float32)
    weight = np.ascontiguousarray(weight, dtype=np.float32)
    bias_b = np.broadcast_to(
        np.ascontiguousarray(bias, dtype=np.float32).reshape(1, OUT_C), (P, OUT_C)
    ).copy()

    nc = _get_program(_rows)
    id16, id32 = _make_aux()
    in_maps = []
    for i in range(NCORES):
        sl = slice(i * _rows, (i + 1) * _rows)
        in_maps.append({
            "H": H[sl], "U": U[sl], "W": weight, "BIASB": bias_b,
            "ID16": id16, "ID32": id32,
        })
    res = run_bass_kernel_spmd(nc, in_maps, core_ids=list(range(NCORES)),
                               trace=_trace)
    out = np.concatenate([res.results[i]["OUT"] for i in range(NCORES)], axis=0)
    if _trace:
        return out, res
    return out



# revision 2
# speedup vs baseline: 1.0313x; 1.0313x over previous
"""Trainium2 Bass kernel for the DJconv hypergraph message-passing layer.

Reference computation (per full input):
    gram = H.T @ H                              [E, E]
    Hu   = concat([H, H @ gram], 1) >= 0.5      [N, 2E] binary
    dv   = Hu.sum(1);  inv = rsqrt(dv) (0 where dv==0)
    out  = ((1 + inv)[:, None] * U) @ weight + bias

Sharding: rows (nodes) split across 8 NeuronCores; the [E, E] gram is
all-gathered + tree-summed on device; weight/bias replicated.

v2 layout:
 - H streamed as fp8e4 (exact for 0/1): DoubleRow gram (half the matmuls),
   fp8 PE transposes (element-step-2 PSUM out) feeding the phase-B
   DoubleRow weights directly.
 - U@W precomputed in bf16 into SBUF during the collective window, so the
   post-collective phase is threshold/epilogue/DMA only.
 - final epilogue fused into one scalar_tensor_tensor per even tile (DVE),
   ACT-mul + Pool-add for odd tiles.
"""

import numpy as np
import ml_dtypes

import concourse.bass as bass
import concourse.tile as tile
from concourse import bacc, mybir
from concourse.bass_utils import run_bass_kernel_spmd

F32 = mybir.dt.float32
BF16 = mybir.dt.bfloat16
FP8 = mybir.dt.float8e4

N_FULL, E, IN_C, OUT_C = 131072, 256, 128, 256
NCORES = 8
ROWS = N_FULL // NCORES          # 16384 rows per core
P = 128
SCALE = 64.0
GW = 272  # 257 padded so the DR rhs slot stride is 16B-aligned


def build_program(rows=ROWS, ncores=NCORES):
    assert rows % 512 == 0
    nt = rows // P          # node tiles per core
    ns = nt // 4            # supertiles (4 node tiles each)

    nc = bacc.Bacc("TRN2", target_bir_lowering=False, debug=False,
                   num_devices=ncores)

    H = nc.dram_tensor("H", [rows, E], F32, kind="ExternalInput").ap()
    U = nc.dram_tensor("U", [rows, IN_C], F32, kind="ExternalInput").ap()
    W16 = nc.dram_tensor("W16", [IN_C, OUT_C], BF16, kind="ExternalInput").ap()
    B16 = nc.dram_tensor("B16", [P, OUT_C], BF16, kind="ExternalInput").ap()
    B32 = nc.dram_tensor("B32", [P, OUT_C], F32, kind="ExternalInput").ap()
    ID8 = nc.dram_tensor("ID8", [P, P], FP8, kind="ExternalInput").ap()
    ID16 = nc.dram_tensor("ID16", [P, P], BF16, kind="ExternalInput").ap()
    OUT = nc.dram_tensor("OUT", [rows, OUT_C], F32, kind="ExternalOutput").ap()

    # node (s*512 + p*4 + j): 4 consecutive rows per partition -> 4KB DMA lines.
    # Same permutation applied to H, U and OUT, so the kernel is self-consistent.
    H_r = H.rearrange("(s p j) e -> s p j e", j=4, p=P)
    U_r = U.rearrange("(s p j) c -> s p j c", j=4, p=P)
    OUT_r = OUT.rearrange("(s p j) o -> s p j o", j=4, p=P)

    with tile.TileContext(nc) as tc:
        _body(tc, nt, ns, H_r, U_r, OUT_r, W16, B16, B32, ID8, ID16)

    nc.compile()
    return nc


def _body(tc, nt, ns, H_r, U_r, OUT_r, W16, B16, B32, ID8, ID16):
    nc = tc.nc
    Add = mybir.AluOpType.add
    Mult = mybir.AluOpType.mult
    IsGe = mybir.AluOpType.is_ge
    AF = mybir.ActivationFunctionType
    DR = mybir.MatmulPerfMode.DoubleRow

    import contextlib
    ctx = contextlib.ExitStack()
    with ctx:
        const = ctx.enter_context(tc.tile_pool(name="const", bufs=1))
        htst = ctx.enter_context(tc.tile_pool(name="htstore", bufs=1))
        work = ctx.enter_context(tc.tile_pool(name="work", bufs=1))
        opool = ctx.enter_context(tc.tile_pool(name="ost", bufs=4))
        scr = ctx.enter_context(tc.tile_pool(name="scratch", bufs=3))
        dram = ctx.enter_context(tc.tile_pool(name="dram", bufs=1, space="DRAM"))

        # ---- constants ----
        id8 = const.tile([P, P], FP8)
        nc.sync.dma_start(id8[:], ID8[:])
        id16 = const.tile([P, P], BF16)
        nc.sync.dma_start(id16[:], ID16[:])
        w_sb = const.tile([IN_C, OUT_C], BF16)
        nc.sync.dma_start(w_sb[:], W16[:])
        bias16 = const.tile([P, OUT_C], BF16)
        nc.sync.dma_start(bias16[:], B16[:])
        bias32 = const.tile([P, OUT_C], F32)
        nc.sync.dma_start(bias32[:], B32[:])
        neghalf = const.tile([P, 1], F32)
        nc.vector.memset(neghalf[:], -0.5 / SCALE)

        # persistent H^T (fp8, DR-packed: slot t holds edges t*128..t*128+127)
        HTE = htst.tile([P, 2, nt * P], FP8, tag="hte")

        gcat = work.tile([P, E + P], BF16, tag="gcat")

        # ---- phase A: stream H as fp8; DR gram + fp8 transposes chase DMA ----
        with tc.tile_pool(name="hallp", bufs=1) as hallp:
            H8 = hallp.tile([P, ns, 4, E], FP8, tag="h8")
            with tc.tile_pool(name="psA", bufs=1, space="PSUM") as psA, \
                 tc.tile_pool(name="psT", bufs=3, space="PSUM") as psT:
                gA = psA.tile([P, E], F32, tag="gA")
                gB = psA.tile([P, P], F32, tag="gB")
                for s in range(ns):
                    nc.gpsimd.dma_start(H8[:, s], H_r[s])   # f32 -> fp8 cast
                    for q in (0, 2):
                        first = (s == 0 and q == 0)
                        last = (s == ns - 1 and q == 2)
                        nc.tensor.matmul(gA[:], H8[:, s, q:q + 2, 0:P],
                                         H8[:, s, q:q + 2, :],
                                         perf_mode=DR, start=first, stop=last)
                        nc.tensor.matmul(gB[:], H8[:, s, q:q + 2, P:E],
                                         H8[:, s, q:q + 2, P:E],
                                         perf_mode=DR, start=first, stop=last)
                    pt0 = psT.tile([P, 4, P, 2], FP8, tag="t0")
                    pt1 = psT.tile([P, 4, P, 2], FP8, tag="t1")
                    for j in range(4):
                        nc.tensor.transpose(pt0[:, j, :, 0], H8[:, s, j, 0:P],
                                            id8[:])
                        nc.tensor.transpose(pt1[:, j, :, 0], H8[:, s, j, P:E],
                                            id8[:])
                    sl = slice(s * 4 * P, (s + 1) * 4 * P)
                    nc.vector.tensor_copy(HTE[:, 0, sl], pt0[:, :, :, 0])
                    nc.scalar.copy(HTE[:, 1, sl], pt1[:, :, :, 0])
                nc.vector.tensor_copy(gcat[:, 0:E], gA[:])
                nc.vector.tensor_copy(gcat[:, E:E + P], gB[:])

        # ---- collective: one AllGather of the bf16 gram partial ----
        cc_in = dram.tile([P, E + P], BF16)
        cc_out = dram.tile([NCORES * P, E + P], BF16)
        nc.sync.dma_start(cc_in[:], gcat[:])
        nc.gpsimd.collective_compute(
            "AllGather", mybir.AluOpType.bypass,
            replica_groups=[list(range(NCORES))],
            ins=[cc_in.opt()],
            outs=[cc_out.opt()],
        )
        gparts = work.tile([P, NCORES, E + P], BF16, tag="gparts")
        nc.sync.dma_start(gparts[:], cc_out[:].rearrange("(r p) f -> p r f", p=P))

        # ---- U staging + U@W precompute in bf16 (fills the collective window)
        uwp = ctx.enter_context(tc.tile_pool(name="uwp", bufs=1))
        UW = uwp.tile([P, nt, OUT_C], BF16, tag="uw")
        with tc.tile_pool(name="uring", bufs=4) as uring, \
             tc.tile_pool(name="utring", bufs=3) as utring, \
             tc.tile_pool(name="psU", bufs=3, space="PSUM") as psU, \
             tc.tile_pool(name="psF", bufs=4, space="PSUM") as psF:
            for s in range(ns):
                u16 = uring.tile([P, 4, IN_C], BF16, tag="u")
                nc.gpsimd.dma_start(u16[:], U_r[s])     # f32 -> bf16 cast
                pp = psU.tile([P, 4, IN_C], BF16, tag="pp")
                for j in range(4):
                    nc.tensor.transpose(pp[:, j], u16[:, j], id16[:])
                utr = utring.tile([P, 4, IN_C], BF16, tag="ut")
                if s % 2 == 0:
                    nc.vector.tensor_copy(utr[:], pp[:])
                else:
                    nc.scalar.copy(utr[:], pp[:])
                for j in range(4):
                    k = 4 * s + j
                    po = psF.tile([P, OUT_C], F32, tag="po")
                    nc.tensor.matmul(po[:], utr[:, j], w_sb[:],
                                     start=True, stop=True)
                    if k % 2 == 0:
                        nc.vector.tensor_copy(UW[:, k], po[:])
                    else:
                        nc.scalar.copy(UW[:, k], po[:])

        # ---- tree-sum the 8 gathered gram partials -> gsum [P, 384] ----
        gsum = work.tile([P, E + P], BF16, tag="gsum")
        g4 = work.tile([P, 4, E + P], BF16, tag="g4")
        nc.vector.tensor_tensor(g4[:], gparts[:, 0:4], gparts[:, 4:8], op=Add)
        g2 = work.tile([P, 2, E + P], BF16, tag="g2")
        nc.vector.tensor_tensor(g2[:], g4[:, 0:2], g4[:, 2:4], op=Add)
        nc.vector.tensor_tensor(gsum[:], g2[:, 0], g2[:, 1], op=Add)

        # ---- gxp: fp8 DR-packed gram (scaled 1/SCALE) + ones column ----
        gxp = const.tile([P, 2, GW], FP8, tag="gxp")
        with tc.tile_pool(name="psG", bufs=1, space="PSUM") as psG:
            nc.vector.memset(gxp[:], 0.0)
            nc.vector.tensor_scalar(gxp[:, 0, 0:E], gsum[:, 0:E], 1.0 / SCALE,
                                    None, op0=Mult)
            nc.vector.tensor_scalar(gxp[:, 1, P:E], gsum[:, E:E + P],
                                    1.0 / SCALE, None, op0=Mult)
            pgt = psG.tile([P, P], BF16, tag="pgt")
            nc.tensor.transpose(pgt[:], gsum[:, P:E], id16[:])
            nc.vector.tensor_scalar(gxp[:, 1, 0:P], pgt[:], 1.0 / SCALE,
                                    None, op0=Mult)
            nc.vector.memset(gxp[:, 0, E:E + 1], 1.0)
            nc.vector.memset(gxp[:, 1, E:E + 1], 1.0)

        # ---- phase B + final epilogue, interleaved in chunks of 32 tiles ----
        dvS = work.tile([P, nt], F32, tag="dvS")
        dvH = work.tile([P, nt], F32, tag="dvH")
        s1p = work.tile([P, nt], F32, tag="s1p")

        def dv_chunk(c0, c1):
            csl = slice(c0, c1)
            # even cols hold 2*cnt-256 (Sign); fix to cnt
            nc.vector.tensor_scalar(dvS[:, c0:c1:2], dvS[:, c0:c1:2], 0.5,
                                    float(E) / 2, op0=Mult, op1=Add)
            dv = work.tile([P, nt], F32, tag="dv")
            nc.vector.tensor_tensor(dv[:, csl], dvS[:, csl], dvH[:, csl], op=Add)
            mx = work.tile([P, nt], F32, tag="mx")
            nc.vector.tensor_scalar_max(mx[:, csl], dv[:, csl], 1.0)
            rc = work.tile([P, nt], F32, tag="rc")
            nc.vector.reciprocal(rc[:, csl], mx[:, csl])
            sq = work.tile([P, nt], F32, tag="sq")
            nc.scalar.sqrt(sq[:, csl], dv[:, csl])
            r0 = work.tile([P, nt], F32, tag="r0")
            nc.vector.tensor_tensor(r0[:, csl], sq[:, csl], rc[:, csl], op=Mult)
            q = work.tile([P, nt], F32, tag="q")
            nc.vector.tensor_tensor(q[:, csl], r0[:, csl], r0[:, csl], op=Mult)
            nc.vector.tensor_tensor(q[:, csl], q[:, csl], dv[:, csl], op=Mult)
            nc.vector.tensor_scalar(q[:, csl], q[:, csl], -0.5, 1.5,
                                    op0=Mult, op1=Add)
            nc.vector.tensor_tensor(s1p[:, csl], r0[:, csl], q[:, csl], op=Mult)
            nc.vector.tensor_scalar_add(s1p[:, csl], s1p[:, csl], 1.0)

        CH = 32
        with tc.tile_pool(name="psB", bufs=6, space="PSUM") as psB:
            for c0 in range(0, nt, CH):
                for k in range(c0, c0 + CH):
                    pb = psB.tile([P, GW], F32, tag="pb")
                    ksl = slice(k * P, (k + 1) * P)
                    nc.tensor.matmul(pb[:], HTE[:, :, ksl], gxp[:],
                                     perf_mode=DR, start=True, stop=True)
                    sg = scr.tile([P, E], BF16, tag="sg")
                    if k % 2 == 0:
                        nc.scalar.activation(sg[:], pb[:, 0:E], AF.Sign,
                                             bias=neghalf[:], scale=1.0,
                                             accum_out=dvS[:, k:k + 1])
                        nc.vector.tensor_copy(dvH[:, k:k + 1], pb[:, E:E + 1])
                    else:
                        nc.vector.tensor_scalar(sg[:], pb[:, 0:E], 0.5 / SCALE,
                                                0.0, op0=IsGe, op1=Add,
                                                accum_out=dvS[:, k:k + 1])
                        nc.scalar.copy(dvH[:, k:k + 1], pb[:, E:E + 1])
                dv_chunk(c0, c0 + CH)
                for s in range(c0 // 4, (c0 + CH) // 4):
                    ob = opool.tile([P, 4, OUT_C], F32, tag="ob")
                    for j in range(4):
                        k = 4 * s + j
                        if k % 2 == 0:
                            nc.vector.scalar_tensor_tensor(
                                ob[:, j], UW[:, k], s1p[:, k:k + 1], bias16[:],
                                op0=Mult, op1=Add)
                        else:
                            ys = scr.tile([P, OUT_C], F32, tag="ys")
                            nc.scalar.mul(ys[:], UW[:, k], s1p[:, k:k + 1])
                            nc.gpsimd.tensor_tensor(ob[:, j], ys[:], bias32[:],
                                                    op=Add)
                    nc.sync.dma_start(OUT_r[s], ob[:])


_CACHE = {}


def _get_program(rows=ROWS):
    if rows not in _CACHE:
        _CACHE[rows] = build_program(rows=rows)
    return _CACHE[rows]


def kernel(H, U, weight, bias, _rows=ROWS, _trace=False):
    H = np.ascontiguousarray(H, dtype=np.float32)
    U = np.ascontiguousarray(U, dtype=np.float32)
    w16 = np.ascontiguousarray(weight, dtype=np.float32).astype(ml_dtypes.bfloat16)
    bias32 = np.broadcast_to(
        np.ascontiguousarray(bias, dtype=np.float32).reshape(1, OUT_C), (P, OUT_C)
    ).copy()
    bias16 = bias32.astype(ml_dtypes.bfloat16)
    id8 = np.eye(P, dtype=mybir.dt.np(FP8))
    id16 = np.eye(P, dtype=ml_dtypes.bfloat16)

    nc = _get_program(_rows)
    in_maps = []
    for i in range(NCORES):
        sl = slice(i * _rows, (i + 1) * _rows)
        in_maps.append({
            "H": H[sl], "U": U[sl], "W16": w16, "B16": bias16, "B32": bias32,
            "ID8": id8, "ID16": id16,
        })
    res = run_bass_kernel_spmd(nc, in_maps, core_ids=list(range(NCORES)),
                               trace=_trace)
    out = np.concatenate([res.results[i]["OUT"] for i in range(NCORES)], axis=0)
    if _trace:
        return out, res
    return out


# revision 3
# speedup vs baseline: 1.0490x; 1.0171x over previous
"""Trainium2 Bass kernel for the DJconv hypergraph message-passing layer.

Reference computation (per full input):
    gram = H.T @ H                              [E, E]
    Hu   = concat([H, H @ gram], 1) >= 0.5      [N, 2E] binary
    dv   = Hu.sum(1);  inv = rsqrt(dv) (0 where dv==0)
    out  = ((1 + inv)[:, None] * U) @ weight + bias

Sharding: rows (nodes) split across 8 NeuronCores; the [E, E] gram partial
is AllReduced (bf16, exact for the 0-vs-nonzero threshold decisions);
weight/bias replicated.

v3 structure:
 - H streamed as fp8e4 (exact 0/1): DoubleRow gram, fp8 PE transposes
   (element-step-2 PSUM) feeding phase-B DoubleRow weights.
 - per-node H rowsums (dvH) computed during phase A on DVE, so phase B has
   no ones-column and no per-tile accumulator copies.
 - U fully prebuffered in bf16 so its DMA issues precede the collective
   trigger on the in-order gpsimd stream; U^T + U@W (bf16) precomputed
   into SBUF during the collective window.
 - post-collective: DoubleRow HG matmuls + thresholds (DVE/ACT split) +
   one fused scale+bias op per tile, output streamed per supertile.
"""

import numpy as np
import ml_dtypes

import concourse.bass as bass
import concourse.tile as tile
from concourse import bacc, mybir
from concourse.bass_utils import run_bass_kernel_spmd

F32 = mybir.dt.float32
BF16 = mybir.dt.bfloat16
FP8 = mybir.dt.float8e4

N_FULL, E, IN_C, OUT_C = 131072, 256, 128, 256
NCORES = 8
ROWS = N_FULL // NCORES          # 16384 rows per core
P = 128
SCALE = 64.0
GW = E  # phase-B matmul width (no ones column needed)


def build_program(rows=ROWS, ncores=NCORES):
    assert rows % 512 == 0
    nt = rows // P          # node tiles per core
    ns = nt // 4            # supertiles (4 node tiles each)

    nc = bacc.Bacc("TRN2", target_bir_lowering=False, debug=False,
                   num_devices=ncores)

    H = nc.dram_tensor("H", [rows, E], F32, kind="ExternalInput").ap()
    U = nc.dram_tensor("U", [rows, IN_C], F32, kind="ExternalInput").ap()
    W16 = nc.dram_tensor("W16", [IN_C, OUT_C], BF16, kind="ExternalInput").ap()
    B16 = nc.dram_tensor("B16", [P, OUT_C], BF16, kind="ExternalInput").ap()
    B32 = nc.dram_tensor("B32", [P, OUT_C], F32, kind="ExternalInput").ap()
    ID8 = nc.dram_tensor("ID8", [P, P], FP8, kind="ExternalInput").ap()
    ID16 = nc.dram_tensor("ID16", [P, P], BF16, kind="ExternalInput").ap()
    OUT = nc.dram_tensor("OUT", [rows, OUT_C], F32, kind="ExternalOutput").ap()

    # node (s*512 + p*4 + j): 4 consecutive rows per partition -> 4KB DMA lines.
    # Same permutation applied to H, U and OUT, so the kernel is self-consistent.
    H_r = H.rearrange("(s p j) e -> s p j e", j=4, p=P)
    U_r = U.rearrange("(s p j) c -> s p j c", j=4, p=P)
    OUT_r = OUT.rearrange("(s p j) o -> s p j o", j=4, p=P)

    with tile.TileContext(nc) as tc:
        _body(tc, nt, ns, H_r, U_r, OUT_r, W16, B16, B32, ID8, ID16)

    nc.compile()
    return nc


def _body(tc, nt, ns, H_r, U_r, OUT_r, W16, B16, B32, ID8, ID16):
    nc = tc.nc
    Add = mybir.AluOpType.add
    Mult = mybir.AluOpType.mult
    IsGe = mybir.AluOpType.is_ge
    AF = mybir.ActivationFunctionType
    DR = mybir.MatmulPerfMode.DoubleRow
    AX = mybir.AxisListType.X

    import contextlib
    ctx = contextlib.ExitStack()
    with ctx:
        const = ctx.enter_context(tc.tile_pool(name="const", bufs=1))
        htst = ctx.enter_context(tc.tile_pool(name="htstore", bufs=1))
        work = ctx.enter_context(tc.tile_pool(name="work", bufs=1))
        ob16p = ctx.enter_context(tc.tile_pool(name="ob16", bufs=3))
        ob32p = ctx.enter_context(tc.tile_pool(name="ob32", bufs=3))
        scr = ctx.enter_context(tc.tile_pool(name="scratch", bufs=3))
        dram = ctx.enter_context(tc.tile_pool(name="dram", bufs=1, space="DRAM"))

        # ---- constants ----
        id8 = const.tile([P, P], FP8)
        nc.sync.dma_start(id8[:], ID8[:])
        id16 = const.tile([P, P], BF16)
        nc.sync.dma_start(id16[:], ID16[:])
        w_sb = const.tile([IN_C, OUT_C], BF16)
        nc.sync.dma_start(w_sb[:], W16[:])
        bias16 = const.tile([P, OUT_C], BF16)
        nc.sync.dma_start(bias16[:], B16[:])
        bias32 = const.tile([P, OUT_C], F32)
        nc.sync.dma_start(bias32[:], B32[:])
        neghalf = const.tile([P, 1], F32)
        nc.vector.memset(neghalf[:], -0.5 / SCALE)

        # persistent H^T (fp8, DR-packed: slot t holds edges t*128..t*128+127)
        HTE = htst.tile([P, 2, nt * P], FP8, tag="hte")
        # U prebuffer (bf16) so all U DMA issues precede the collective trigger
        U16 = htst.tile([P, ns, 4, IN_C], BF16, tag="u16")

        dvH = work.tile([P, nt], F32, tag="dvH")   # rowsum(H) per node
        gcat = work.tile([P, E + P], BF16, tag="gcat")

        # ---- phase A: stream H as fp8; DR gram + fp8 transposes chase DMA ----
        with tc.tile_pool(name="hallp", bufs=1) as hallp:
            H8 = hallp.tile([P, ns, 4, E], FP8, tag="h8")
            with tc.tile_pool(name="psA", bufs=1, space="PSUM") as psA, \
                 tc.tile_pool(name="psT", bufs=3, space="PSUM") as psT:
                gA = psA.tile([P, E], F32, tag="gA")
                gB = psA.tile([P, P], F32, tag="gB")
                for s in range(ns):
                    nc.gpsimd.dma_start(H8[:, s], H_r[s])   # f32 -> fp8 cast
                    for q in (0, 2):
                        first = (s == 0 and q == 0)
                        last = (s == ns - 1 and q == 2)
                        nc.tensor.matmul(gA[:], H8[:, s, q:q + 2, 0:P],
                                         H8[:, s, q:q + 2, :],
                                         perf_mode=DR, start=first, stop=last)
                        nc.tensor.matmul(gB[:], H8[:, s, q:q + 2, P:E],
                                         H8[:, s, q:q + 2, P:E],
                                         perf_mode=DR, start=first, stop=last)
                    pt0 = psT.tile([P, 4, P, 2], FP8, tag="t0")
                    pt1 = psT.tile([P, 4, P, 2], FP8, tag="t1")
                    for j in range(4):
                        nc.tensor.transpose(pt0[:, j, :, 0], H8[:, s, j, 0:P],
                                            id8[:])
                        nc.tensor.transpose(pt1[:, j, :, 0], H8[:, s, j, P:E],
                                            id8[:])
                    sl = slice(s * 4 * P, (s + 1) * 4 * P)
                    nc.scalar.copy(HTE[:, 0, sl], pt0[:, :, :, 0])
                    nc.scalar.copy(HTE[:, 1, sl], pt1[:, :, :, 0])
                    # per-node H rowsums for dv (DVE, fp8 source)
                    nc.vector.tensor_reduce(dvH[:, 4 * s:4 * s + 4], H8[:, s],
                                            axis=AX, op=Add)
                nc.vector.tensor_copy(gcat[:, 0:E], gA[:])
                nc.vector.tensor_copy(gcat[:, E:E + P], gB[:])

        # ---- U DMA issues (gpsimd stream, before the collective trigger) ----
        for s in range(ns):
            nc.gpsimd.dma_start(U16[:, s], U_r[s])      # f32 -> bf16 cast

        # ---- collective: AllReduce the bf16 gram partial ----
        cc_in = dram.tile([P, E + P], BF16)
        cc_out = dram.tile([P, E + P], BF16)
        nc.sync.dma_start(cc_in[:], gcat[:])
        nc.gpsimd.collective_compute(
            "AllReduce", Add,
            replica_groups=[list(range(NCORES))],
            ins=[cc_in.opt()],
            outs=[cc_out.opt()],
        )
        gsum = work.tile([P, E + P], BF16, tag="gsum")
        nc.sync.dma_start(gsum[:], cc_out[:])

        # ---- U^T + U@W precompute in bf16 (fills the collective window) ----
        uwp = ctx.enter_context(tc.tile_pool(name="uwp", bufs=1))
        UW = uwp.tile([P, nt, OUT_C], BF16, tag="uw")
        with tc.tile_pool(name="utring", bufs=3) as utring, \
             tc.tile_pool(name="psU", bufs=3, space="PSUM") as psU, \
             tc.tile_pool(name="psF", bufs=4, space="PSUM") as psF:
            for s in range(ns):
                pp = psU.tile([P, 4, IN_C], BF16, tag="pp")
                for j in range(4):
                    nc.tensor.transpose(pp[:, j], U16[:, s, j], id16[:])
                utr = utring.tile([P, 4, IN_C], BF16, tag="ut")
                if s % 2 == 0:
                    nc.vector.tensor_copy(utr[:], pp[:])
                else:
                    nc.scalar.copy(utr[:], pp[:])
                for j in range(4):
                    k = 4 * s + j
                    po = psF.tile([P, OUT_C], F32, tag="po")
                    nc.tensor.matmul(po[:], utr[:, j], w_sb[:],
                                     start=True, stop=True)
                    if k % 2 == 0:
                        nc.vector.tensor_copy(UW[:, k], po[:])
                    else:
                        nc.scalar.copy(UW[:, k], po[:])

        # ---- gxp: fp8 DR-packed gram (scaled 1/SCALE) ----
        gxp = const.tile([P, 2, GW], FP8, tag="gxp")
        with tc.tile_pool(name="psG", bufs=1, space="PSUM") as psG:
            nc.vector.tensor_scalar(gxp[:, 0, :], gsum[:, 0:E], 1.0 / SCALE,
                                    None, op0=Mult)
            nc.vector.tensor_scalar(gxp[:, 1, P:E], gsum[:, E:E + P],
                                    1.0 / SCALE, None, op0=Mult)
            pgt = psG.tile([P, P], BF16, tag="pgt")
            nc.tensor.transpose(pgt[:], gsum[:, P:E], id16[:])
            nc.vector.tensor_scalar(gxp[:, 1, 0:P], pgt[:], 1.0 / SCALE,
                                    None, op0=Mult)

        # ---- phase B + final epilogue, interleaved in chunks of 16 tiles ----
        dvS = work.tile([P, nt], F32, tag="dvS")
        s1p = work.tile([P, nt], F32, tag="s1p")     # 1 + rsqrt(dv)
        s1p16 = work.tile([P, nt], BF16, tag="s1p16")
        CH = 16
        NACT = 6   # tiles per chunk thresholded on ACT (contiguous tail)

        def dv_chunk(c0, c1):
            csl = slice(c0, c1)
            # ACT-thresholded tail holds 2*cnt-256 (Sign); fix to cnt
            nc.vector.tensor_scalar(dvS[:, c1 - NACT:c1], dvS[:, c1 - NACT:c1],
                                    0.5, float(E) / 2, op0=Mult, op1=Add)
            dv = work.tile([P, nt], F32, tag="dv")
            nc.vector.tensor_tensor(dv[:, csl], dvS[:, csl], dvH[:, csl], op=Add)
            mx = work.tile([P, nt], F32, tag="mx")
            nc.vector.tensor_scalar_max(mx[:, csl], dv[:, csl], 1.0)
            rc = work.tile([P, nt], F32, tag="rc")
            nc.vector.reciprocal(rc[:, csl], mx[:, csl])
            sq = work.tile([P, nt], F32, tag="sq")
            nc.scalar.sqrt(sq[:, csl], dv[:, csl])
            r0 = work.tile([P, nt], F32, tag="r0")
            nc.vector.tensor_tensor(r0[:, csl], sq[:, csl], rc[:, csl], op=Mult)
            q = work.tile([P, nt], F32, tag="q")
            nc.vector.tensor_tensor(q[:, csl], r0[:, csl], r0[:, csl], op=Mult)
            nc.vector.tensor_tensor(q[:, csl], q[:, csl], dv[:, csl], op=Mult)
            nc.vector.tensor_scalar(q[:, csl], q[:, csl], -0.5, 1.5,
                                    op0=Mult, op1=Add)
            nc.vector.tensor_tensor(s1p[:, csl], r0[:, csl], q[:, csl], op=Mult)
            nc.vector.tensor_scalar_add(s1p[:, csl], s1p[:, csl], 1.0)
            nc.vector.tensor_copy(s1p16[:, csl], s1p[:, csl])

        with tc.tile_pool(name="psB", bufs=6, space="PSUM") as psB:
            for c0 in range(0, nt, CH):
                for k in range(c0, c0 + CH):
                    pb = psB.tile([P, GW], F32, tag="pb")
                    ksl = slice(k * P, (k + 1) * P)
                    nc.tensor.matmul(pb[:], HTE[:, :, ksl], gxp[:],
                                     perf_mode=DR, start=True, stop=True)
                    sg = scr.tile([P, E], BF16, tag="sg")
                    if k - c0 < CH - NACT:
                        nc.vector.tensor_scalar(sg[:], pb[:], 0.5 / SCALE,
                                                0.0, op0=IsGe, op1=Add,
                                                accum_out=dvS[:, k:k + 1])
                    else:
                        nc.scalar.activation(sg[:], pb[:], AF.Sign,
                                             bias=neghalf[:], scale=1.0,
                                             accum_out=dvS[:, k:k + 1])
                dv_chunk(c0, c0 + CH)
                for s in range(c0 // 4, (c0 + CH) // 4):
                    if s % 2 == 0:
                        # all-bf16 fused scale+bias on DVE; casting OUT DMA
                        ob = ob16p.tile([P, 4, OUT_C], BF16, tag="ob16")
                        for j in range(4):
                            k = 4 * s + j
                            nc.vector.scalar_tensor_tensor(
                                ob[:, j], UW[:, k], s1p16[:, k:k + 1],
                                bias16[:], op0=Mult, op1=Add)
                        nc.gpsimd.dma_start(OUT_r[s], ob[:])  # bf16 -> f32
                    else:
                        ob = ob32p.tile([P, 4, OUT_C], F32, tag="ob32")
                        for j in range(4):
                            k = 4 * s + j
                            ys = scr.tile([P, OUT_C], F32, tag="ys")
                            nc.scalar.mul(ys[:], UW[:, k], s1p[:, k:k + 1])
                            nc.gpsimd.tensor_tensor(ob[:, j], ys[:], bias32[:],
                                                    op=Add)
                        nc.sync.dma_start(OUT_r[s], ob[:])


_CACHE = {}


def _get_program(rows=ROWS):
    if rows not in _CACHE:
        _CACHE[rows] = build_program(rows=rows)
    return _CACHE[rows]


def kernel(H, U, weight, bias, _rows=ROWS, _trace=False):
    H = np.ascontiguousarray(H, dtype=np.float32)
    U = np.ascontiguousarray(U, dtype=np.float32)
    w16 = np.ascontiguousarray(weight, dtype=np.float32).astype(ml_dtypes.bfloat16)
    bias32 = np.broadcast_to(
        np.ascontiguousarray(bias, dtype=np.float32).reshape(1, OUT_C), (P, OUT_C)
    ).copy()
    bias16 = bias32.astype(ml_dtypes.bfloat16)
    id8 = np.eye(P, dtype=mybir.dt.np(FP8))
    id16 = np.eye(P, dtype=ml_dtypes.bfloat16)

    nc = _get_program(_rows)
    in_maps = []
    for i in range(NCORES):
        sl = slice(i * _rows, (i + 1) * _rows)
        in_maps.append({
            "H": H[sl], "U": U[sl], "W16": w16, "B16": bias16, "B32": bias32,
            "ID8": id8, "ID16": id16,
        })
    res = run_bass_kernel_spmd(nc, in_maps, core_ids=list(range(NCORES)),
                               trace=_trace)
    out = np.concatenate([res.results[i]["OUT"] for i in range(NCORES)], axis=0)
    if _trace:
        return out, res
    return out
